# revision 2
# baseline (speedup 1.0000x reference)
"""DLRM (embedding_lookup) Trainium2 Bass kernel.

Layout strategy (data-parallel over batch, tables replicated per core):
  - Each of the 8 cores handles S = B/8 = 2048 samples end-to-end.
  - Embedding gather: one indirect DMA per 128-sample tile (128x26 rows of
    256B each) straight from the replicated tables in device DRAM.
  - All activations are kept feature-major ([features, samples]) so every
    matmul is a plain lhsT.T @ rhs with no transposes between layers.
  - Pairwise-dot interaction: per-4-sample packed gram matmuls on the PE
    (tokens in fp16), followed by a PSUM->SBUF diagonal stripe extraction,
    per-u PE transposes into a flat per-sample gram, and a final set of PE
    transposes that produce the interaction features pair-major so the top
    MLP consumes them directly.  The top MLP folds the upper-triangular
    selection into a host-precomputed symmetric weight W'' (exact: 0.5 on
    both (t,u) and (u,t), zero diagonal).
"""

import os
import sys

import numpy as np

if "/opt/trn_rl_repo" not in sys.path:
    sys.path.insert(0, "/opt/trn_rl_repo")
os.environ.setdefault("JAX_PLATFORMS", "cpu")

B = 16384
NCORES = 8
ND = 13
F = 26
V1 = 131073  # rows per table (V + 1)
D = 64
T = 27  # 1 + F tokens
NPAIR = T * (T - 1) // 2  # 351
BOT0, BOT1 = 128, 64
TOP0, TOP1 = 512, 256


# ----------------------------------------------------------------------------
# host-side weight / index prep
# ----------------------------------------------------------------------------

def prep_host(inputs, vrows_per_table=V1, f=F):
    """Transform full inputs into the tensors the device kernel wants."""
    t = 1 + f
    d = D
    sparse = np.asarray(inputs["sparse"])
    emb = np.ascontiguousarray(np.asarray(inputs["emb_tables"], np.float32))
    nf = emb.shape[0]
    assert nf == f and emb.shape[1] == vrows_per_table and emb.shape[2] == d
    flat_idx = (
        np.arange(f, dtype=np.int64)[None, :] * vrows_per_table
        + sparse.astype(np.int64)
    ).astype(np.int32)

    w_t1 = np.asarray(inputs["w_t1"], np.float32)  # [D + npair, 512]
    npair = t * (t - 1) // 2
    assert w_t1.shape[0] == d + npair
    w1d = np.ascontiguousarray(w_t1[:d])  # [64, 512]
    w1p = w_t1[d:]  # [npair, 512]
    iu0, iu1 = np.triu_indices(t, k=1)
    wpp_full = np.zeros((t * t, w_t1.shape[1]), np.float32)
    wpp_full[iu0 * t + iu1] = 0.5 * w1p
    wpp_full[iu1 * t + iu0] = 0.5 * w1p
    # pack k-tiles of 128 rows side by side: wpp[p, kb*512 + j]
    nkb = (t * t + 127) // 128
    wpp = np.zeros((128, nkb * TOP0), np.float32)
    for kb in range(nkb):
        rows = wpp_full[kb * 128 : (kb + 1) * 128]
        wpp[: rows.shape[0], kb * TOP0 : kb * TOP0 + TOP0] = rows

    w2 = np.asarray(inputs["w_t2"], np.float32)  # [512, 256]
    w2p = np.zeros((128, 4 * TOP1), np.float32)
    for k in range(4):
        w2p[:, k * TOP1 : (k + 1) * TOP1] = w2[k * 128 : (k + 1) * 128]
    wo = np.asarray(inputs["w_o"], np.float32)  # [256, 1]
    wop = np.zeros((128, 2), np.float32)
    wop[:, 0] = wo[:128, 0]
    wop[:, 1] = wo[128:, 0]
    bt1 = np.asarray(inputs["b_t1"], np.float32).reshape(4, 128).T.copy()  # [128,4]
    bt2 = np.asarray(inputs["b_t2"], np.float32).reshape(2, 128).T.copy()  # [128,2]

    return dict(
        dense=np.ascontiguousarray(np.asarray(inputs["dense"], np.float32)),
        fidx=np.ascontiguousarray(flat_idx),
        emb=np.ascontiguousarray(emb.reshape(f * vrows_per_table, d)),
        wb1=np.ascontiguousarray(np.asarray(inputs["w_b1"], np.float32)),  # [13,128]
        bb1=np.asarray(inputs["b_b1"], np.float32).reshape(BOT0, 1).copy(),
        wb2=np.ascontiguousarray(np.asarray(inputs["w_b2"], np.float32)),  # [128,64]
        bb2=np.asarray(inputs["b_b2"], np.float32).reshape(BOT1, 1).copy(),
        w1d=w1d,
        wpp=wpp,
        bt1=np.ascontiguousarray(bt1),
        w2p=w2p,
        bt2=np.ascontiguousarray(bt2),
        wop=wop,
        bo=float(np.asarray(inputs["b_o"]).reshape(-1)[0]),
    )


# ----------------------------------------------------------------------------
# device kernel emission
# ----------------------------------------------------------------------------

def emit_dlrm(tc, outs, ins, S, f=F, bo=0.0, debug_dumps=False):
    """Emit the per-core DLRM program into TileContext tc.

    ins: dict of APs  (dense [S,13], fidx [S,f], emb [f*V, 64], weights...)
    outs: dict with 'logit' [1, S]
    """
    import concourse.bass as bass
    import concourse.mybir as mybir
    from concourse.masks import make_identity

    nc = tc.nc
    t = 1 + f
    f32 = mybir.dt.float32
    f32r = mybir.dt.float32r
    f16 = mybir.dt.float16

    NT = S // 128  # sample tiles
    NG = S // 4  # groups of 4 samples (= class-local sample count)
    NCH = min(512, NG)  # top-mlp N chunk
    NKB = (t * t + 127) // 128  # 6 k-tiles over the flat gram
    LASTK = t * t - (NKB - 1) * 128  # rows in last gram k-tile (89)
    GBANKS = NT * 8  # one gram psum bank per 4 groups (16 samples)

    ident_n = 128

    cp = tc.alloc_tile_pool(name="const", bufs=1)
    ident = cp.tile([ident_n, ident_n], f32)
    make_identity(nc, ident)
    ident16 = cp.tile([ident_n, ident_n], f16)
    nc.vector.tensor_copy(out=ident16, in_=ident)
    bo_s = cp.tile([1, 1], f32)
    nc.gpsimd.memset(bo_s, bo)

    wb1_s = cp.tile_from(ins["wb1"])    # [13, 128]
    bb1_s = cp.tile_from(ins["bb1"])    # [128, 1]
    wb2_s = cp.tile_from(ins["wb2"])    # [128, 64]
    bb2_s = cp.tile_from(ins["bb2"])    # [64, 1]
    w1d_s = cp.tile_from(ins["w1d"])    # [64, 512]
    wpp_s = cp.tile_from(ins["wpp"])    # [128, NKB*512]
    bt1_s = cp.tile_from(ins["bt1"])    # [128, 4]
    w2p_s = cp.tile_from(ins["w2p"])    # [128, 4*256]
    bt2_s = cp.tile_from(ins["bt2"])    # [128, 2]
    wop_s = cp.tile_from(ins["wop"])    # [128, 2]
    w1d_r = cp.tile([D, TOP0], f32r)
    nc.vector.tensor_copy(out=w1d_r, in_=w1d_s)
    wpp_r = cp.tile(list(wpp_s.shape), f32r)
    nc.vector.tensor_copy(out=wpp_r, in_=wpp_s)
    w2p_r = cp.tile([128, 4 * TOP1], f32r)
    nc.vector.tensor_copy(out=w2p_r, in_=w2p_s)
    wop_r = cp.tile([128, 2], f32r)
    nc.vector.tensor_copy(out=wop_r, in_=wop_s)
    ident_r = cp.tile([ident_n, ident_n], f32r)
    nc.vector.tensor_copy(out=ident_r, in_=ident)

    # persistent activations
    pp = tc.alloc_tile_pool(name="persist", bufs=1)
    GR = pp.tile([128, NG * t], f32r)         # [ (c-band*32 + t) , (g, u) ]
    nc.gpsimd.memset(GR.bitcast(f32), 0.0)
    dtok = pp.tile([64, S], f32)              # dense token, feature-major
    dtok_r = pp.tile([64, S], f32r)
    dtok16 = pp.tile([64, S], f16)
    XG = pp.tile([128, 4 * NKB * NG], f32r)   # per class: k-chunk-major gram fm
    if debug_dumps:
        nc.gpsimd.memset(XG.bitcast(f32), 0.0)
    logit_sb = pp.tile([1, S], f32)

    emb_ap = ins["emb"]
    relu = mybir.ActivationFunctionType.Relu
    ident_fn = mybir.ActivationFunctionType.Identity

    # warmup: the very first indirect DMA issued corrupts its first row on
    # hardware; sacrifice one dummy gather so real ones are clean.
    zidx = cp.tile([128, 1], mybir.dt.int32)
    nc.gpsimd.memset(zidx, 0)
    scratch = cp.tile([128, D], f32)
    nc.gpsimd.indirect_dma_start(
        out=scratch,
        out_offset=None,
        in_=emb_ap,
        in_offset=bass.IndirectOffsetOnAxis(ap=zidx, axis=0),
    )

    # ---------------- per-tile phase ----------------
    with (
        tc.tile_pool(name="io", bufs=3) as iop,
        tc.tile_pool(name="embp", bufs=2) as embp,
        tc.tile_pool(name="tokp", bufs=2) as tokp,
        tc.tile_pool(name="botps", bufs=2, space="PSUM") as botps,
        tc.tile_pool(name="tokps", bufs=2, space="PSUM") as tokps,
        tc.tile_pool(name="gps", bufs=2, space="PSUM") as gps,
    ):
        for it in range(NT):
            s0 = it * 128
            dense_t = iop.tile([128, ND], f32, tag="dense")
            nc.sync.dma_start(out=dense_t, in_=ins["dense"][s0 : s0 + 128, :])
            fidx_t = iop.tile([128, f], mybir.dt.int32, tag="fidx")
            nc.sync.dma_start(out=fidx_t, in_=ins["fidx"][s0 : s0 + 128, :])

            emb_t = embp.tile([128, f * D], f32, tag="emb")
            emb_tv = emb_t.rearrange("p (f d) -> p f d", d=D)
            for ff in range(f):
                nc.gpsimd.indirect_dma_start(
                    out=emb_tv[:, ff, :],
                    out_offset=None,
                    in_=emb_ap,
                    in_offset=bass.IndirectOffsetOnAxis(
                        ap=fidx_t[:, ff : ff + 1], axis=0
                    ),
                )
            if debug_dumps and it == 0:
                nc.sync.dma_start(out=outs["d_emb"], in_=emb_t)
            # cast fp32 -> fp16 for the PE transposes / gram matmuls
            emb16 = embp.tile([128, f * D], f16, tag="emb16")
            nc.vector.tensor_copy(out=emb16[:, : f * D // 2], in_=emb_t[:, : f * D // 2])
            nc.scalar.copy(out=emb16[:, f * D // 2 :], in_=emb_t[:, f * D // 2 :])

            # ---- bottom MLP (feature-major) ----
            dps = botps.tile([ND, 128], f32, tag="bot")
            nc.tensor.transpose(out=dps, in_=dense_t, identity=ident)
            dT = iop.tile([ND, 128], f32, tag="dT")
            nc.vector.tensor_copy(out=dT, in_=dps)
            h1ps = botps.tile([BOT0, 128], f32, tag="bot")
            nc.tensor.matmul(out=h1ps, lhsT=wb1_s, rhs=dT, start=True, stop=True)
            h1b = iop.tile([BOT0, 128], f32, tag="h1b")
            nc.scalar.activation(out=h1b, in_=h1ps, func=relu, bias=bb1_s, scale=1.0)
            dtps = botps.tile([BOT1, 128], f32, tag="bot")
            nc.tensor.matmul(out=dtps, lhsT=wb2_s, rhs=h1b, start=True, stop=True)
            nc.scalar.activation(
                out=dtok[:, s0 : s0 + 128], in_=dtps, func=relu, bias=bb2_s, scale=1.0
            )
            nc.vector.tensor_copy(
                out=dtok16[:, s0 : s0 + 128], in_=dtok[:, s0 : s0 + 128]
            )
            nc.scalar.copy(
                out=dtok_r[:, s0 : s0 + 128], in_=dtok[:, s0 : s0 + 128]
            )

            # ---- token matrix TOK [64, 128*32] fp16, sample-major:
            #      col = s_local*32 + tslot  (tslots 27..31 zero pad) ----
            TOK = tokp.tile([64, 128 * 32], f16, tag="tok")
            tok3 = TOK.rearrange("p (s t) -> p s t", t=32)
            nc.vector.memset(tok3[:, :, t:], 0.0)
            nc.vector.tensor_copy(
                out=tok3[:, :, 0], in_=dtok16[:, s0 : s0 + 128]
            )
            emb3 = emb16.rearrange("p (f d) -> p f d", d=D)
            half = 13
            for chunk in range(2):
                tle = range(chunk * half, f) if chunk else range(half)
                tps = tokps.tile([64, half * 128], f16, tag="tokps")
                for j, tt in enumerate(tle):
                    nc.tensor.transpose(
                        out=tps[:, j * 128 : (j + 1) * 128],
                        in_=emb3[:, tt, :],
                        identity=ident16,
                    )
                n = len(tle)
                # dst: token tt+1 of each sample -> col s*32 + (tt+1)
                dst = tok3[:, :, 1 + chunk * half : 1 + chunk * half + n].rearrange(
                    "p s t -> p t s"
                )
                src = tps[:, : n * 128].rearrange("p (t s) -> p t s", t=n)
                if chunk == 0:
                    nc.vector.tensor_copy(out=dst, in_=src)
                else:
                    nc.scalar.copy(out=dst, in_=src)

            # ---- gram packs: 8 psum banks x 4 groups of 4 samples ----
            for bk in range(8):
                gp = gps.tile([128, 512], f32, tag="g")
                for g in range(4):
                    loc = (bk * 4 + g) * 4  # local sample offset of this group
                    opnd = TOK[:, loc * 32 : loc * 32 + 128]
                    nc.tensor.matmul(
                        out=gp[:, g * 128 : (g + 1) * 128],
                        lhsT=opnd,
                        rhs=opnd,
                        start=True,
                        stop=True,
                    )
                # stripe extraction: diag class-bands -> GR (bands at 32c)
                g0 = it * 32 + bk * 4  # global group index of this bank
                gp3 = gp.rearrange("p (g w) -> p g w", g=4)
                gr3 = GR.rearrange("p (s u) -> p s u", u=t)
                for c in range(4):
                    src = gp3[c * 32 : c * 32 + t, :, c * 32 : c * 32 + t]
                    dst = gr3[c * 32 : c * 32 + t, g0 : g0 + 4, :]
                    if c < 2:
                        nc.vector.tensor_copy(out=dst, in_=src)
                    else:
                        nc.scalar.copy(out=dst, in_=src)

    # ---------------- gram reshape phase ----------------
    # GR [(c,t) , (s', u)] -> FGQ [s'chunk , (c, t, u)] -> XG [(t,u)-chunk, s']
    ublocks = [(u0, min(4, t - u0)) for u0 in range(0, t, 4)]
    with (
        tc.tile_pool(name="fgq", bufs=1) as fgqp,
        tc.tile_pool(name="ups", bufs=2, space="PSUM") as upsp,
        tc.tile_pool(name="xps", bufs=2, space="PSUM") as xpsp,
    ):
        gr3 = GR.rearrange("p (s u) -> p u s", u=t)
        for sc in range(NG // 128):
            FGQ = fgqp.tile([128, 4 * t * t], f32r, tag="fgq")
            for u0, un in ublocks:
                ups = upsp.tile([128, 4 * 128], f32r, tag="ups")
                for j in range(un):
                    nc.tensor.transpose(
                        out=ups[:, j * 128 : (j + 1) * 128],
                        in_=gr3[:, u0 + j, sc * 128 : (sc + 1) * 128],
                        identity=ident_r,
                    )
                # interleave copy: src order [j][c][tt] -> FGQ col c*t*t + tt*t + (u0+j)
                src = ups[:, : un * 128].rearrange(
                    "p (j c w) -> p j c w", j=un, c=4
                )[:, :, :, :t]
                dst = FGQ.rearrange("p (c q u) -> p c q u", c=4, q=t)[
                    :, :, :, u0 : u0 + un
                ].rearrange("p c q u -> p u c q")
                if u0 % 8 == 0:
                    nc.vector.tensor_copy(out=dst, in_=src)
                else:
                    nc.scalar.copy(out=dst, in_=src)
            # fm transposes: per class, 6 k-chunks of the 729-wide flat gram
            for c in range(4):
                for kb0 in range(0, NKB, 3):
                    kbs = [kb for kb in range(kb0, min(NKB, kb0 + 3))]
                    xps = xpsp.tile([128, 3 * 128], f32r, tag="xps")
                    for j, kb in enumerate(kbs):
                        kn = 128 if kb < NKB - 1 else LASTK
                        nc.tensor.transpose(
                            out=xps[:kn, j * 128 : j * 128 + 128],
                            in_=FGQ[:, c * t * t + kb * 128 : c * t * t + kb * 128 + kn],
                            identity=ident_r,
                        )
                    xg4 = XG.rearrange("p (c k s) -> p c k s", c=4, k=NKB)
                    for j, kb in enumerate(kbs):
                        kn = 128 if kb < NKB - 1 else LASTK
                        eng = nc.vector if (kb % 2 == 0) else nc.scalar
                        if kb % 2 == 0:
                            eng.tensor_copy(
                                out=xg4[:kn, c, kb, sc * 128 : (sc + 1) * 128],
                                in_=xps[:kn, j * 128 : (j + 1) * 128],
                            )
                        else:
                            eng.copy(
                                out=xg4[:kn, c, kb, sc * 128 : (sc + 1) * 128],
                                in_=xps[:kn, j * 128 : (j + 1) * 128],
                            )

    # ---------------- top MLP (per class, fp32r matmuls) ----------------
    with (
        tc.tile_pool(name="h1p", bufs=2) as h1p,
        tc.tile_pool(name="h2p", bufs=2) as h2p,
        tc.tile_pool(name="topps", bufs=2, space="PSUM") as topps,
        tc.tile_pool(name="outps", bufs=2, space="PSUM") as outps,
    ):
        dtok4 = dtok_r.rearrange("p (s c) -> p c s", c=4)
        xg4 = XG.rearrange("p (c k s) -> p c k s", c=4, k=NKB)
        for c in range(4):
            for n0 in range(0, NG, NCH):
                h1s = h1p.tile([128, 4 * NCH], f32r, tag="h1")
                for m in range(4):
                    hps = topps.tile([128, NCH], f32, tag="h")
                    nc.tensor.matmul(
                        out=hps,
                        lhsT=w1d_r[:, m * 128 : (m + 1) * 128],
                        rhs=dtok4[:, c, n0 : n0 + NCH],
                        start=True,
                        stop=False,
                    )
                    for kb in range(NKB):
                        kn = 128 if kb < NKB - 1 else LASTK
                        nc.tensor.matmul(
                            out=hps,
                            lhsT=wpp_r[:kn, kb * TOP0 + m * 128 : kb * TOP0 + (m + 1) * 128],
                            rhs=xg4[:kn, c, kb, n0 : n0 + NCH],
                            start=False,
                            stop=(kb == NKB - 1),
                        )
                    nc.scalar.activation(
                        out=h1s[:, m * NCH : (m + 1) * NCH],
                        in_=hps,
                        func=relu,
                        bias=bt1_s[:, m : m + 1],
                        scale=1.0,
                    )
                h2s = h2p.tile([128, 2 * NCH], f32r, tag="h2")
                for m in range(2):
                    hps = topps.tile([128, NCH], f32, tag="h")
                    for k in range(4):
                        nc.tensor.matmul(
                            out=hps,
                            lhsT=w2p_r[:, k * TOP1 + m * 128 : k * TOP1 + (m + 1) * 128],
                            rhs=h1s[:, k * NCH : (k + 1) * NCH],
                            start=(k == 0),
                            stop=(k == 3),
                        )
                    nc.scalar.activation(
                        out=h2s[:, m * NCH : (m + 1) * NCH],
                        in_=hps,
                        func=relu,
                        bias=bt2_s[:, m : m + 1],
                        scale=1.0,
                    )
                ops = outps.tile([1, NCH], f32, tag="o")
                for k in range(2):
                    nc.tensor.matmul(
                        out=ops,
                        lhsT=wop_r[:, k : k + 1],
                        rhs=h2s[:, k * NCH : (k + 1) * NCH],
                        start=(k == 0),
                        stop=(k == 1),
                    )
                dst = logit_sb.rearrange("p (s c) -> p c s", c=4)[
                    :, c, n0 : n0 + NCH
                ]
                nc.scalar.activation(
                    out=dst, in_=ops, func=ident_fn, bias=bo_s, scale=1.0
                )

    if debug_dumps:
        nc.sync.dma_start(out=outs["d_dtok"], in_=dtok)
        nc.sync.dma_start(out=outs["d_gr"], in_=GR.bitcast(f32))
        nc.sync.dma_start(out=outs["d_xg"], in_=XG.bitcast(f32))
    nc.sync.dma_start(out=outs["logit"], in_=logit_sb)
    pp.release()
    cp.release()


# ----------------------------------------------------------------------------
# numpy mirror (same math incl. fp16 token rounding) for debugging
# ----------------------------------------------------------------------------

def mirror(prepped, S, f=F):
    t = 1 + f
    dense = prepped["dense"]
    emb = prepped["emb"]
    fidx = prepped["fidx"]
    n = dense.shape[0]
    h = np.maximum(dense @ prepped["wb1"] + prepped["bb1"][:, 0], 0.0)
    dt_ = np.maximum(h @ prepped["wb2"] + prepped["bb2"][:, 0], 0.0)
    toks = np.concatenate([dt_[:, None, :], emb[fidx]], axis=1)  # [n, t, 64]
    toks16 = toks.astype(np.float16).astype(np.float32)
    gram = np.einsum("ntd,nsd->nts", toks16, toks16).reshape(n, t * t)
    nkb = (t * t + 127) // 128
    wpp_full = np.zeros((nkb * 128, TOP0), np.float32)
    for kb in range(nkb):
        wpp_full[kb * 128 : (kb + 1) * 128] = prepped["wpp"][
            :, kb * TOP0 : (kb + 1) * TOP0
        ]
    x1 = dt_ @ prepped["w1d"] + gram @ wpp_full[: t * t]
    bt1 = prepped["bt1"].T.reshape(-1)
    h1 = np.maximum(x1 + bt1, 0.0)
    w2 = np.concatenate(
        [prepped["w2p"][:, k * TOP1 : (k + 1) * TOP1] for k in range(4)], axis=0
    )
    bt2 = prepped["bt2"].T.reshape(-1)
    h2 = np.maximum(h1 @ w2 + bt2, 0.0)
    wo = np.concatenate([prepped["wop"][:, 0], prepped["wop"][:, 1]])
    return h2 @ wo + prepped["bo"]


# ----------------------------------------------------------------------------
# program build + run
# ----------------------------------------------------------------------------

def build_program(S, vrows_total, f=F, bo=0.0, debug_dumps=False):
    import concourse.mybir as mybir
    import concourse.tile as tile
    from concourse import bacc

    nc = bacc.Bacc("TRN2", debug=False, num_devices=1)
    f32 = mybir.dt.float32
    shapes = dict(
        dense=([S, ND], f32),
        fidx=([S, f], mybir.dt.int32),
        emb=([vrows_total, D], f32),
        wb1=([ND, BOT0], f32),
        bb1=([BOT0, 1], f32),
        wb2=([BOT0, BOT1], f32),
        bb2=([BOT1, 1], f32),
        w1d=([D, TOP0], f32),
        wpp=([128, ((1 + f) ** 2 + 127) // 128 * TOP0], f32),
        bt1=([128, 4], f32),
        w2p=([128, 4 * TOP1], f32),
        bt2=([128, 2], f32),
        wop=([128, 2], f32),
    )
    ins = {
        k: nc.dram_tensor(k, shp, dt, kind="ExternalInput").ap()
        for k, (shp, dt) in shapes.items()
    }
    outs = {
        "logit": nc.dram_tensor("logit", [1, S], f32, kind="ExternalOutput").ap()
    }
    if debug_dumps:
        t = 1 + f
        NG = S // 4
        NKB = (t * t + 127) // 128
        outs["d_emb"] = nc.dram_tensor("d_emb", [128, f * D], f32, kind="ExternalOutput").ap()
        outs["d_dtok"] = nc.dram_tensor("d_dtok", [64, S], f32, kind="ExternalOutput").ap()
        outs["d_gr"] = nc.dram_tensor("d_gr", [128, NG * t], f32, kind="ExternalOutput").ap()
        outs["d_xg"] = nc.dram_tensor("d_xg", [128, 4 * NKB * NG], f32, kind="ExternalOutput").ap()
    with tile.TileContext(nc) as tc:
        emit_dlrm(tc, outs, ins, S, f=f, bo=bo, debug_dumps=debug_dumps)
    nc.compile()
    return nc


def make_in_maps(prepped, S):
    shared = {
        k: prepped[k]
        for k in ("emb", "wb1", "bb1", "wb2", "bb2", "w1d", "wpp", "bt1", "w2p", "bt2", "wop")
    }
    in_maps = []
    for c in range(NCORES):
        m = dict(shared)
        m["dense"] = prepped["dense"][c * S : (c + 1) * S]
        m["fidx"] = prepped["fidx"][c * S : (c + 1) * S]
        in_maps.append(m)
    return in_maps


def kernel(**inputs):
    from concourse.bass_utils import run_bass_kernel_spmd

    prepped = prep_host(inputs)
    S = B // NCORES
    nc = build_program(S, F * V1, bo=prepped["bo"])
    in_maps = make_in_maps(prepped, S)
    res = run_bass_kernel_spmd(nc, in_maps, core_ids=list(range(NCORES)))
    out = np.concatenate([res.results[c]["logit"].reshape(-1) for c in range(NCORES)])
    return out.astype(np.float32)

